# revision 17
# baseline (speedup 1.0000x reference)
"""Soft-DTW loss kernel for Trainium2 (Bass/Tile), 8-core data-parallel.

Problem: B=64 samples; per sample cost C = cdist(pred_b, target_b) (512x512),
then soft-DTW DP (gamma=1) over C; loss = mean_b(dtw_b / 1024).

Strategy (v2)
-------------
Data-parallel: 8 samples per core. Per core the DP runs in the exp domain
with smooth per-cell scaling:
  E'[i,j] = E[i,j]*e^{k(i+j)} via EC' = exp(-C+k) and
  v~[j] = E'[i-1,j] + e^k*E'[i-1,j-1]  (one scalar_tensor_tensor)
  row scan: E'[i,j] = (v~[j] + E'[i,j-1]) * EC'[i,j]
so no per-step rescale/clamp ops are needed (k = rhat/1024 from a trace fit).

Meet-in-the-middle: the DP is split into two independent 256-row halves --
forward on rows 1..256 and backward on the 180-degree-rotated cost matrix
(rows 512..257) -- combined by z = sum_j F[256,j]*(B[256,513-j]+B[256,512-j]).
The two wavefronts' instruction streams are interleaved op-by-op on DVE so
the ~95ns dependency latency of one stream hides under the other's execution
(measured exec-bound 138ns/op vs 233ns/op chained).

Each stream: 8 column chunks of W=64 per sample (partition 16b+s, ghost for
the boundary), skewed j-major EC buffer, 135 two-row wavefront steps. The
64-column chunks are 128-slab aligned so each (sample, slab, stream) scatter
is ONE DMA using flat mixed strides (partition pitch + skew), 64 DMAs total.
sqrt/exp(+bias k) run per sample on the Activation engine before the scatter,
overlapped with the next sample's matmuls.
Final: loss_b = (1024*k_b - ln z_b)/1024, reduced on host during the gather.
"""

import numpy as np
from contextlib import ExitStack

import concourse.bass as bass
import concourse.tile as tile
from concourse import bacc, mybir
from concourse.bass_utils import run_bass_kernel_spmd

f32 = mybir.dt.float32
AL = mybir.AluOpType
AF = mybir.ActivationFunctionType

B, S, F = 64, 512, 128
NCORES = 8
BL = B // NCORES          # 8 samples per core
MID = S // 2              # 256 rows per stream
W = 64                    # chunk width
NS = 8                    # chunks per sample per stream
JP = 272                  # per-column pitch in ec buffers (256 rows + 16 skew)
ECL = W * JP              # ec elements per partition (per stream)
CW = W + 1                # slot block width
SLOT = 2 * CW             # ring slot: [c0|d0(64)|c1|d1(64)]
NSTEP = MID // 2 + NS - 1  # 135 two-row wavefront steps
RT = S // 128             # 4 row tiles per sample
# rhat = TR_A * trace(C) + TR_B (offline fit); kappa = rhat/1024 per cell
TR_A = 0.7264
TR_B = 2168.3


def build_core_program(debug_outputs=False):
    nc = bacc.Bacc("TRN2", target_bir_lowering=False, debug=False,
                   num_devices=NCORES)
    pred_d = nc.dram_tensor("pred", [BL, S, F], f32, kind="ExternalInput")
    targ_d = nc.dram_tensor("target", [BL, S, F], f32, kind="ExternalInput")
    zf_d = nc.dram_tensor("zf", [1, BL], f32, kind="ExternalOutput")
    rhat_d = nc.dram_tensor("rhat", [1, BL], f32, kind="ExternalOutput")
    if debug_outputs:
        cf_d = nc.dram_tensor("cf_dbg", [64, W], f32, kind="ExternalOutput")
        cb_d = nc.dram_tensor("cb_dbg", [64, W], f32, kind="ExternalOutput")
        rcb_d = nc.dram_tensor("rcb_dbg", [64, W + 1], f32, kind="ExternalOutput")
        fff_d = nc.dram_tensor("fff_dbg", [128, NS * W], f32, kind="ExternalOutput")
        ffb_d = nc.dram_tensor("ffb_dbg", [128, NS * W], f32, kind="ExternalOutput")
        ecf_d = nc.dram_tensor("ecf_dbg", [128, ECL], f32, kind="ExternalOutput")
        ecb_d = nc.dram_tensor("ecb_dbg", [128, ECL], f32, kind="ExternalOutput")

    with tile.TileContext(nc) as tc, ExitStack() as ctx:
        pool = ctx.enter_context(tc.tile_pool(name="persist", bufs=1))
        spool = ctx.enter_context(tc.tile_pool(name="stage", bufs=2))
        ppool = ctx.enter_context(tc.tile_pool(name="psum", bufs=2, space="PSUM"))
        ppool_t = ctx.enter_context(tc.tile_pool(name="psum_t", bufs=2, space="PSUM"))
        ppool_s = ctx.enter_context(tc.tile_pool(name="psum_small", bufs=1, space="PSUM"))

        # ---------------- persistent tiles ----------------
        ecf = pool.tile([128, ECL], f32, tag="ecf")
        ecb = pool.tile([128, ECL], f32, tag="ecb")
        zrf = pool.tile([128, 3, SLOT], f32, tag="zrf")
        zrb = pool.tile([128, 3, SLOT], f32, tag="zrb")
        vtf = pool.tile([128, W], f32, tag="vtf")
        vtb = pool.tile([128, W], f32, tag="vtb")
        fff = pool.tile([128, NS, W], f32, tag="fff")
        ffb = pool.tile([128, NS, W], f32, tag="ffb")
        ident = pool.tile([128, 128], f32, tag="ident")
        ones = pool.tile([128, 1], f32, tag="ones")
        kall = pool.tile([1, BL], f32, tag="kall")
        rhat_t = pool.tile([1, BL], f32, tag="rhat")
        bmask16 = pool.tile([128, BL], f32, tag="bmask16")  # p//16 == b
        kbc = pool.tile([128, BL], f32, tag="kbc")
        gsel = pool.tile([128, BL], f32, tag="gsel")
        k128 = pool.tile([128, 1], f32, tag="k128")
        g128 = pool.tile([128, 1], f32, tag="g128")         # e^{k_b} per part.
        selb = pool.tile([64, BL], f32, tag="selb")         # p//8 == b
        gsel64 = pool.tile([64, BL], f32, tag="gsel64")
        k64 = pool.tile([64, 1], f32, tag="k64")
        gneg64 = pool.tile([64, 1], f32, tag="gneg64")      # e^{-k_b}
        cf = pool.tile([64, W], f32, tag="cf")
        cb = pool.tile([64, W], f32, tag="cb")
        rcb = pool.tile([64, W + 1], f32, tag="rcb")
        itile = pool.tile([64, W], f32, tag="itile")
        dump64 = pool.tile([64, W], f32, tag="dump64")
        zvec = pool.tile([64, 1], f32, tag="zvec")
        zfin = pool.tile([1, BL], f32, tag="zfin")

        # ---------------- constants / init ----------------
        # big ec zero-fills first so they overlap the first samples' compute
        nc.gpsimd.memset(ecf[:], 0.0)
        nc.gpsimd.memset(ecb[:], 0.0)
        nc.gpsimd.memset(zrf[:], 0.0)
        nc.gpsimd.memset(zrb[:], 0.0)
        nc.gpsimd.memset(vtf[:], 0.0)
        nc.gpsimd.memset(vtb[:], 0.0)
        nc.gpsimd.memset(ones[:], 1.0)

        from concourse import masks
        masks.make_identity(nc, ident[:])

        # bmask16[p, b] = 1 where p//16 == b
        nc.gpsimd.memset(bmask16[:], 1.0)
        nc.gpsimd.affine_select(
            out=bmask16[:], in_=bmask16[:], compare_op=AL.is_ge, fill=0.0,
            base=0, pattern=[[-16, BL]], channel_multiplier=1)
        nc.gpsimd.affine_select(
            out=bmask16[:], in_=bmask16[:], compare_op=AL.is_ge, fill=0.0,
            base=15, pattern=[[16, BL]], channel_multiplier=-1)
        # not7[p] = 0 where p % 8 == 7 else 1 (boundary mask for the combine)
        is7 = pool.tile([64, 8], f32, tag="is7")
        not7 = pool.tile([64, 1], f32, tag="not7")
        nc.gpsimd.memset(is7[:], 0.0)
        nc.gpsimd.affine_select(
            out=is7[:], in_=is7[:], compare_op=AL.not_equal, fill=1.0,
            base=-7, pattern=[[-8, 8]], channel_multiplier=1)
        nc.vector.tensor_reduce(not7[:], is7[:], axis=mybir.AxisListType.X,
                                op=AL.add)
        nc.vector.tensor_scalar(not7[:], not7[:], -1.0, 1.0,
                                op0=AL.mult, op1=AL.add)
        # selb[p, b] = 1 where p//8 == b (64 partitions)
        nc.gpsimd.memset(selb[:], 1.0)
        nc.gpsimd.affine_select(
            out=selb[:], in_=selb[:], compare_op=AL.is_ge, fill=0.0,
            base=0, pattern=[[-8, BL]], channel_multiplier=1)
        nc.gpsimd.affine_select(
            out=selb[:], in_=selb[:], compare_op=AL.is_ge, fill=0.0,
            base=7, pattern=[[8, BL]], channel_multiplier=-1)

        # DP corner seeds: E'[0,0]=1 arrives via prev(slot0) c1 at t=1.
        # fwd chunk 1 at partitions p%16==1; bwd chunk 1 at p%16==8.
        seedf = pool.tile([128, BL], f32, tag="seedf")
        nc.gpsimd.memset(seedf[:], 0.0)
        nc.gpsimd.affine_select(
            out=seedf[:], in_=seedf[:], compare_op=AL.not_equal, fill=1.0,
            base=-1, pattern=[[-16, BL]], channel_multiplier=1)
        nc.vector.tensor_reduce(zrf[:, 0, CW : CW + 1], seedf[:],
                                axis=mybir.AxisListType.X, op=AL.add)
        seedb = pool.tile([128, BL], f32, tag="seedb")
        nc.gpsimd.memset(seedb[:], 0.0)
        nc.gpsimd.affine_select(
            out=seedb[:], in_=seedb[:], compare_op=AL.not_equal, fill=1.0,
            base=-8, pattern=[[-16, BL]], channel_multiplier=1)
        nc.vector.tensor_reduce(zrb[:, 0, CW : CW + 1], seedb[:],
                                axis=mybir.AxisListType.X, op=AL.add)

        # ================= bulk phase: cost -> EC' (both streams) ===========
        RTS = RT * S  # d2s flat partition pitch
        scatter_q = [nc.sync, nc.gpsimd, nc.scalar, nc.sync,
                     nc.gpsimd, nc.scalar, nc.sync, nc.gpsimd]
        for b in range(BL):
            pn = spool.tile([128, RT, F], f32, tag="pn")
            tn = spool.tile([128, RT, F], f32, tag="tn")
            nc.sync.dma_start(pn[:], pred_d[b].rearrange("(a p) f -> p a f", p=128))
            nc.sync.dma_start(tn[:], targ_d[b].rearrange("(a p) f -> p a f", p=128))

            ttr = spool.tile([128, RT, 128], f32, tag="ttr")   # -2 * target^T
            ptr = spool.tile([128, RT, 128], f32, tag="ptr")   # pred^T
            for rt in range(RT):
                ps = ppool_t.tile([128, 128], f32, tag="pst")
                nc.tensor.matmul(ps[:], tn[:, rt], ident[:],
                                 start=True, stop=True, is_transpose=True)
                nc.scalar.activation(ttr[:, rt], ps[:], AF.Copy, scale=-2.0)
                ps2 = ppool_t.tile([128, 128], f32, tag="pst")
                nc.tensor.matmul(ps2[:], pn[:, rt], ident[:],
                                 start=True, stop=True, is_transpose=True)
                nc.scalar.copy(ptr[:, rt], ps2[:])

            # y2[p, ct] = sum_f target^2 (per target col = d2s partition)
            y2 = spool.tile([128, RT], f32, tag="y2")
            dump = spool.tile([128, F], f32, tag="dump")
            for ct in range(RT):
                nc.vector.scalar_tensor_tensor(
                    dump[:], tn[:, ct], 1.0, tn[:, ct],
                    op0=AL.mult, op1=AL.mult, accum_out=y2[:, ct : ct + 1])
            # x2 flat [1, 512] via ones-matmul over (pred^T)^2
            tsq = spool.tile([128, RT * 128], f32, tag="tsq")
            ptr_flat = ptr[:].rearrange("p a f -> p (a f)")
            nc.vector.tensor_mul(tsq[:], ptr_flat, ptr_flat)
            x2p = ppool_s.tile([1, S], f32, tag="x2p")
            nc.tensor.matmul(x2p[:], ones[:], tsq[:], start=True, stop=True)
            x2s = spool.tile([1, S], f32, tag="x2s")
            nc.scalar.copy(x2s[:], x2p[:])
            x2b = spool.tile([128, S], f32, tag="x2b")
            nc.gpsimd.partition_broadcast(x2b[:], x2s[:])

            d2s = spool.tile([128, RT, S], f32, tag="d2s")  # d2T: [c, ct, r]
            qd = spool.tile([128, RT], f32, tag="qd")
            for ct in range(RT):
                mm = ppool.tile([128, S], f32, tag="mm")
                nc.tensor.matmul(mm[:], ttr[:, ct], ptr_flat,
                                 start=True, stop=True)
                nc.vector.scalar_tensor_tensor(
                    d2s[:, ct], mm[:], y2[:, ct : ct + 1], x2b[:],
                    op0=AL.add, op1=AL.add)
                nc.vector.scalar_tensor_tensor(
                    dump[:, 0:128], d2s[:, ct, ct * 128 : (ct + 1) * 128], 1.0,
                    ident[:], op0=AL.mult, op1=AL.mult,
                    accum_out=qd[:, ct : ct + 1])

            # trace -> kappa_b; fold e^{kappa} into the exp as a bias
            nc.scalar.activation(qd[:], qd[:], AF.Sqrt)
            trp = ppool_s.tile([1, RT], f32, tag="trp")
            nc.tensor.matmul(trp[:], ones[:], qd[:], start=True, stop=True)
            trs = spool.tile([1, RT], f32, tag="trs")
            nc.scalar.copy(trs[:], trp[:])
            kb = spool.tile([1, 1], f32, tag="kb")
            nc.vector.tensor_reduce(kb[:], trs[:], axis=mybir.AxisListType.X,
                                    op=AL.add)
            nc.vector.tensor_scalar(kb[:], kb[:], TR_A / 1024.0, TR_B / 1024.0,
                                    op0=AL.mult, op1=AL.add)
            nc.vector.tensor_copy(kall[:, b : b + 1], kb[:])
            kb_bc = spool.tile([128, 1], f32, tag="kb_bc")
            nc.gpsimd.partition_broadcast(kb_bc[:], kb[:])

            d2f = d2s[:].rearrange("p a r -> p (a r)")
            nc.scalar.activation(d2f, d2f, AF.Sqrt)
            nc.scalar.activation(d2f, d2f, AF.Exp, scale=-1.0, bias=kb_bc[:])

            # scatter: one DMA per (slab-half, stream). fwd chunk s=1+2a+k at
            # partition 16b+s, skew 2s; bwd buffer keeps source col order
            # (chunk s'=8-2a-k at the same partition, skew 16-2s'=4a+2k); the
            # bwd scans read it with a negative J stride instead.
            pitch = ecf[:].ap[0][0]
            qi = 0
            for a in range(RT):
                for k in range(2):
                    p = 1 + 2 * a + k
                    basef = ecf[16 * b + p : 16 * b + p + 1, 0:1]
                    dstf = bass.AP(basef.tensor, basef.offset + 2 * p,
                                   [[pitch, 1], [JP, W], [1, MID]])
                    scatter_q[qi % len(scatter_q)].dma_start(
                        dstf, d2s[64 * k : 64 * k + W, a, 0:MID])
                    qi += 1
                    baseb = ecb[16 * b + p : 16 * b + p + 1, 0:1]
                    dstb = bass.AP(baseb.tensor, baseb.offset + 4 * a + 2 * k,
                                   [[pitch, 1], [JP, W], [1, MID]])
                    scatter_q[qi % len(scatter_q)].dma_start(
                        dstb, d2s[64 * k : 64 * k + W, a, MID:S])
                    qi += 1

        # ---- per-partition scale factors ----
        nc.gpsimd.partition_broadcast(kbc[:], kall[:])
        nc.vector.tensor_mul(gsel[:], kbc[:], bmask16[:])
        nc.vector.tensor_reduce(k128[:], gsel[:], axis=mybir.AxisListType.X,
                                op=AL.add)
        nc.scalar.activation(g128[:], k128[:], AF.Exp)
        nc.vector.tensor_mul(gsel64[:], kbc[0:64, :], selb[:])
        nc.vector.tensor_reduce(k64[:], gsel64[:], axis=mybir.AxisListType.X,
                                op=AL.add)
        nc.scalar.activation(gneg64[:], k64[:], AF.Exp, scale=-1.0)
        nc.vector.tensor_scalar(rhat_t[:], kall[:], 1024.0, 0.0,
                                op0=AL.mult, op1=AL.add)

        # ================= interleaved fwd/bwd wavefronts ===================
        shuf_f = [(i - 1 if 1 <= (i % 16) <= 8 else i) for i in range(32)]
        shuf_b = [(i + 1 if (i % 16) <= 8 else i) for i in range(32)]

        def ec_ap(ec, off, rev=False):
            a0 = ec[:]
            return bass.AP(a0.tensor, a0.offset + off,
                           [[ECL, 128], [-JP if rev else JP, W]])

        for t in range(1, NSTEP + 1):
            cur, prv = t % 3, (t - 1) % 3
            nc.vector.stream_shuffle(
                zrf[:, cur, 0 : SLOT : CW], zrf[:, prv, W : SLOT : CW], shuf_f)
            nc.vector.stream_shuffle(
                zrb[:, cur, 0 : SLOT : CW], zrb[:, prv, W : SLOT : CW], shuf_b)
            nc.vector.scalar_tensor_tensor(
                vtf[:], zrf[:, prv, CW : SLOT - 1], g128[:],
                zrf[:, prv, CW + 1 : SLOT], op0=AL.mult, op1=AL.add)
            nc.vector.scalar_tensor_tensor(
                vtb[:], zrb[:, prv, CW : SLOT - 1], g128[:],
                zrb[:, prv, CW + 1 : SLOT], op0=AL.mult, op1=AL.add)
            nc.vector.tensor_tensor_scan(
                zrf[:, cur, 1 : CW], vtf[:], ec_ap(ecf, 2 * t),
                zrf[:, cur, 0 : 1], op0=AL.add, op1=AL.mult)
            nc.vector.tensor_tensor_scan(
                zrb[:, cur, 1 : CW], vtb[:],
                ec_ap(ecb, (W - 1) * JP + 271 - 2 * t, rev=True),
                zrb[:, cur, 0 : 1], op0=AL.add, op1=AL.mult)
            nc.vector.scalar_tensor_tensor(
                vtf[:], zrf[:, cur, 0 : CW - 1], g128[:],
                zrf[:, cur, 1 : CW], op0=AL.mult, op1=AL.add)
            nc.vector.scalar_tensor_tensor(
                vtb[:], zrb[:, cur, 0 : CW - 1], g128[:],
                zrb[:, cur, 1 : CW], op0=AL.mult, op1=AL.add)
            nc.vector.tensor_tensor_scan(
                zrf[:, cur, CW + 1 : SLOT], vtf[:], ec_ap(ecf, 2 * t + 1),
                zrf[:, cur, CW : CW + 1], op0=AL.add, op1=AL.mult)
            nc.vector.tensor_tensor_scan(
                zrb[:, cur, CW + 1 : SLOT], vtb[:],
                ec_ap(ecb, (W - 1) * JP + 270 - 2 * t, rev=True),
                zrb[:, cur, CW : CW + 1], op0=AL.add, op1=AL.mult)
            if t >= NSTEP - NS + 1:  # t in [128, 135]: chunk t-127 finishes
                blk = t - (NSTEP - NS + 1)
                nc.vector.tensor_copy(fff[:, blk], zrf[:, cur, CW + 1 : SLOT])
                nc.vector.tensor_copy(ffb[:, blk], zrb[:, cur, CW + 1 : SLOT])

        # ================= compact + combine =================
        # cf[8b+k, j] = fff[16b+k+1, k, j]  (F'[256, 64k+j+1])
        # crt[8b+k, j] = ffb[16b+k+1, 7-k, j]; rcb[8b+k, j] = crt[8b+k, 63-j]
        #   = B'[256, 513-(64k+j+1)]
        fffap, ffbap = fff[:], ffb[:]
        cfap, cbap = cf[:], cb[:]
        compact_q = [nc.sync, nc.gpsimd, nc.scalar]
        for k in range(NS):
            compact_q[k % 3].dma_start(
                bass.AP(cfap.tensor, cfap.offset + k * W,
                        [[NS * W, BL], [1, W]]),
                bass.AP(fffap.tensor,
                        fffap.offset + (k + 1) * NS * W + k * W,
                        [[16 * NS * W, BL], [1, W]]))
            compact_q[(k + 1) % 3].dma_start(
                bass.AP(cbap.tensor, cbap.offset + k * W,
                        [[NS * W, BL], [1, W]]),
                bass.AP(ffbap.tensor,
                        ffbap.offset + (k + 1) * NS * W + (NS - 1 - k) * W,
                        [[16 * NS * W, BL], [1, W]]))
        cbr = cb[:]
        nc.vector.tensor_copy(
            rcb[:, 0:W],
            bass.AP(cbr.tensor, cbr.offset + W - 1, [[cbr.ap[0][0], 64], [-1, W]]))
        masknext = [((i + 1) if (i % 8) != 7 else i) for i in range(32)]
        nc.vector.stream_shuffle(rcb[:, W : W + 1], rcb[:, 0:1], masknext)
        nc.vector.tensor_mul(rcb[:, W : W + 1], rcb[:, W : W + 1], not7[:])
        # inner = e^{-k} * B'[513-c] + B'[512-c]
        nc.vector.scalar_tensor_tensor(
            itile[:], rcb[:, 0:W], gneg64[:], rcb[:, 1 : W + 1],
            op0=AL.mult, op1=AL.add)
        # z partial per partition, then sum the 8 chunks of each sample
        nc.vector.scalar_tensor_tensor(
            dump64[:], itile[:], 1.0, cf[:], op0=AL.mult, op1=AL.mult,
            accum_out=zvec[:])
        zp = ppool_s.tile([1, BL], f32, tag="zp")
        nc.tensor.matmul(zp[:], zvec[:], selb[:], start=True, stop=True)
        nc.vector.tensor_copy(zfin[:], zp[:])
        nc.sync.dma_start(zf_d[:, :], zfin[:])
        nc.sync.dma_start(rhat_d[:, :], rhat_t[:])
        if debug_outputs:
            nc.sync.dma_start(cf_d[:, :], cf[:])
            nc.sync.dma_start(cb_d[:, :], cb[:])
            nc.sync.dma_start(rcb_d[:, :], rcb[:])
            nc.sync.dma_start(fff_d[:, :], fff[:].rearrange("p a w -> p (a w)"))
            nc.sync.dma_start(ffb_d[:, :], ffb[:].rearrange("p a w -> p (a w)"))
            nc.sync.dma_start(ecf_d[:, :], ecf[:])
            nc.sync.dma_start(ecb_d[:, :], ecb[:])

    nc.compile()
    return nc


_NC_CACHE = {}


def _get_nc(debug_outputs=False):
    key = bool(debug_outputs)
    if key not in _NC_CACHE:
        _NC_CACHE[key] = build_core_program(debug_outputs=key)
    return _NC_CACHE[key]


def kernel(pred, target, _debug=False):
    pred = np.asarray(pred, dtype=np.float32)
    target = np.asarray(target, dtype=np.float32)
    nc = _get_nc(False)
    in_maps = []
    for c in range(NCORES):
        sl = slice(c * BL, (c + 1) * BL)
        in_maps.append({"pred": np.ascontiguousarray(pred[sl]),
                        "target": np.ascontiguousarray(target[sl])})
    res = run_bass_kernel_spmd(nc, in_maps, list(range(NCORES)))
    zf = np.concatenate([res.results[c]["zf"][0] for c in range(NCORES)])
    rhat = np.concatenate([res.results[c]["rhat"][0] for c in range(NCORES)])
    losses = (rhat.astype(np.float64) - np.log(zf.astype(np.float64))) / 1024.0
    if _debug:
        return np.float32(losses.mean()), {"z": zf, "rhat": rhat, "losses": losses}
    return np.float32(losses.mean())


if __name__ == "__main__":
    rng = np.random.default_rng(0)
    p = rng.standard_normal((B, S, F)).astype(np.float32)
    t = rng.standard_normal((B, S, F)).astype(np.float32)
    out, dbg = kernel(p, t, _debug=True)
    print("loss:", out)
    print("z:", dbg["z"][:8])
    print("losses:", dbg["losses"][:8])
